# revision 26
# baseline (speedup 1.0000x reference)
"""DVAE encoder (batched DAG GRU message passing) on 8 trn2 NeuronCores.

Strategy: pure data-parallel over batch (256 graphs/core). Feature-major
compute with fp8-e4m3 DoubleRow matmuls (2 K-tiles per instruction) for the
GRU/gate/mapper contractions. Scale scheme (all recovered by exact pow-2
activation scales): messages carry 16x, GRU weights 8x on h-columns and
128x on x/bias columns, gate/mapper weights 16x. The predecessor
aggregation runs message-stationary on the PE: lhsT = message pairs,
moving = host-prepped diagonal adjacency-mask pairs, so the aggregate is
produced directly in feature-major PSUM (no batch-major detour). Per-step
gate/mapper bias rows, one-hot x rows, and the const-1 row are streamed by
DMA into fp8 weight/rhs rows, off all compute engines. Pointwise work is
split at half-tile granularity across DVE / GpSimd / Scalar so the 16-step
recurrence pipelines.
"""

import numpy as np

B, MAX_N, NVT, HS, NZ = 2048, 16, 8, 501, 56
NC_CORES = 8
BL = B // NC_CORES   # 256 graphs per core
SM = 16.0            # message scale (msb carries SM*gm; agg psum = SM*h_in)
SWH = 8.0            # GRU weight scale on h-columns
SAB = 1.0 / (SWH * SM)   # GRU activation scale
SG = 16.0            # gate/mapper weight scale

_CACHE = {}


def _mask_plan():
    """Aggregation schedule: per (vn, bt) a diagonal mask per predecessor u.
    u < vn-1 are issued early (prefix); u = vn-1 after msb[vn-1] is written.
    Returns ({(vn, bt): [(mask_idx, u)]}, nmasks)."""
    out = {}
    nm = 0
    for vn in range(1, MAX_N):
        for bt in range(2):
            ent = []
            for u in range(vn):
                ent.append((nm, u))
                nm += 1
            out[(vn, bt)] = ent
    return out, nm


def _build_nc():
    import concourse.mybir as mybir
    import concourse.tile as tile
    from concourse import bacc

    F32 = mybir.dt.float32
    BF = mybir.dt.bfloat16
    F8 = mybir.dt.float8e4
    DR = mybir.MatmulPerfMode.DoubleRow
    SIG = mybir.ActivationFunctionType.Sigmoid
    TANH = mybir.ActivationFunctionType.Tanh
    IDENT = mybir.ActivationFunctionType.Identity
    MUL = mybir.AluOpType.mult
    SUB = mybir.AluOpType.subtract

    plan, NMSK = _mask_plan()

    nc = bacc.Bacc("TRN2", target_bir_lowering=False, debug=False,
                   num_devices=NC_CORES)

    d_wrz = nc.dram_tensor("wrz", [128, 4096], F8, kind="ExternalInput").ap()
    d_wc = nc.dram_tensor("wc", [128, 4096], F8, kind="ExternalInput").ap()
    d_wb = nc.dram_tensor("wb", [128, 1024], F8, kind="ExternalInput").ap()
    d_wgm = nc.dram_tensor("wgm", [128, 8192], F8, kind="ExternalInput").ap()
    d_wf = nc.dram_tensor("wf", [128, 448], BF, kind="ExternalInput").ap()
    d_fcb = nc.dram_tensor("fcb", [128, 1], F32, kind="ExternalInput").ap()
    d_xh = nc.dram_tensor("xh", [128, MAX_N * 256], F8,
                          kind="ExternalInput").ap()
    d_gbr = nc.dram_tensor("gbr", [1, MAX_N * 2048], F8,
                           kind="ExternalInput").ap()
    d_one = nc.dram_tensor("onerow", [1, 256], F8, kind="ExternalInput").ap()
    d_msk = nc.dram_tensor("msk", [128, NMSK * 128], BF,
                           kind="ExternalInput").ap()
    d_id16 = nc.dram_tensor("id16", [128, 128], BF,
                            kind="ExternalInput").ap()
    d_y = nc.dram_tensor("y", [112, BL], F32, kind="ExternalOutput").ap()
    import os
    DBG = os.environ.get("KDBG", "0") == "1"
    DBGV = int(os.environ.get("KDBG_V", "0"))
    if DBG:
        d_dbg = {}
        for nm, shp, dt in (("dbg_ht0", [128, 1024], F8),
                            ("dbg_rs", [128, 1024], BF),
                            ("dbg_ns", [128, 512], BF),
                            ("dbg_hv", [128, 1024], F8),
                            ("dbg_gm", [128, 512], BF),
                            ("dbg_msb", [128, 1024], BF),
                            ("dbg_agg", [128, 1024], F32),
                            ("dbg_ht1", [128, 1024], F8)):
            d_dbg[nm] = nc.dram_tensor(nm, shp, dt,
                                       kind="ExternalOutput").ap()

    with tile.TileContext(nc) as tc:
        with tc.tile_pool(name="wts", bufs=1) as wts, \
             tc.tile_pool(name="state", bufs=1) as state, \
             tc.tile_pool(name="hio", bufs=2) as hio, \
             tc.tile_pool(name="work", bufs=2) as work, \
             tc.tile_pool(name="psA", bufs=2, space="PSUM") as psA, \
             tc.tile_pool(name="pagg", bufs=1, space="PSUM") as paggp:

            wrz = wts.tile([128, 2, 2, 4, 2, 128], F8, tag="wrz", name="wrz")
            wc = wts.tile([128, 4, 4, 2, 128], F8, tag="wc", name="wc")
            wb = wts.tile([128, 2, 4, 128], F8, tag="wb", name="wb")
            wf = wts.tile([128, 4, 112], BF, tag="wf", name="wf")
            fcb = wts.tile([128, 1], F32, tag="fcb", name="fcb")
            id16 = wts.tile([128, 128], BF, tag="id16", name="id16")
            msk = wts.tile([128, NMSK, 128], BF, tag="msk", name="msk")
            for t, d in ((wrz, d_wrz), (wc, d_wc), (wb, d_wb), (wf, d_wf),
                         (fcb, d_fcb), (id16, d_id16), (msk, d_msk)):
                nc.sync.dma_start(out=t[:], in_=d[:])

            # mutable weight tile (per-step bias rows DMA'd into row 127)
            wgm = state.tile([128, 4, 2, 4, 2, 128], F8, tag="wgm",
                             name="wgm")
            nc.sync.dma_start(out=wgm[:], in_=d_wgm[:])
            # step-0 gate/mapper bias rows (later steps stream inside loop)
            nc.sync.dma_start(out=wgm[127:, 1, :, :, 1, :],
                              in_=d_gbr[:, 0:1024])
            nc.sync.dma_start(out=wgm[127:, 3, :, :, 1, :],
                              in_=d_gbr[:, 1024:2048])
            # messages, batch-major: [128b, u, bt, 512f] (SM-scaled fp8)
            msb = state.tile([128, MAX_N, 2, 512], BF, tag="msb", name="msb")

            def new_hT(v):
                # rhs tile [128, kp, ki, 256] for the GRU contraction of
                # step v; x one-hot + GRU bias + zero rows (117:128 of
                # k-tile 3) are DMA'd from the host-prepped table.
                hT = hio.tile([128, 2, 2, 256], F8, tag="hT", name="hT")
                nc.sync.dma_start(out=hT[117:128, 1, 1, :],
                                  in_=d_xh[117:128, v * 256:(v + 1) * 256])
                return hT

            hT = hio.tile([128, 2, 2, 256], F8, tag="hT", name="hT")
            nc.vector.memset(hT[:], 0.0)
            nc.sync.dma_start(out=hT[117:128, 1, 1, :],
                              in_=d_xh[117:128, 0:256])

            agg_prev = None
            for v in range(MAX_N):
                vn = v + 1
                if DBG and v == DBGV:
                    nc.sync.dma_start(out=d_dbg["dbg_ht0"], in_=hT[:])
                # ---- phase A matmuls ----
                rz_ps, cb_ps = [], []
                for h in range(2):
                    p = psA.tile([128, 2, 2, 256], F32, tag="ps", name="rzp")
                    rz_ps.append(p)
                    for rz in range(2):
                        for mt2 in range(2):
                            mt = 2 * h + mt2
                            for kp in range(2):
                                nc.tensor.matmul(
                                    p[:, rz, mt2, :],
                                    wrz[:, kp, rz, mt, :, :],
                                    hT[:, kp, :, :],
                                    start=(kp == 0), stop=(kp == 1),
                                    perf_mode=DR)
                for h in range(2):
                    p = psA.tile([128, 2, 256], F32, tag="cb", name="cbp",
                                 bufs=2)
                    cb_ps.append(p)
                    for mt2 in range(2):
                        mt = 2 * h + mt2
                        for kp in range(4):
                            nc.tensor.matmul(
                                p[:, mt2, :], wc[:, kp, mt, :, :],
                                hT[:, kp % 2, :, :],
                                start=(kp == 0), stop=(kp == 3),
                                perf_mode=DR)

                # ---- GRU pointwise ----
                rzs, n_sb, d_sb = [], [], []
                if v < MAX_N - 1:
                    hv8 = work.tile([128, 2, 2, 256], F8, tag="hv8",
                                    name="hv8")
                else:
                    hv16 = work.tile([128, 4, 256], BF, tag="hv16",
                                     name="hv16")
                for h in range(2):
                    rs = work.tile([128, 2, 2, 256], BF, tag=f"rzs{h}",
                                   name="rzs")
                    rzs.append(rs)
                    nc.scalar.activation(rs[:], rz_ps[h][:], SIG, scale=SAB)
                    # pre_n built in PSUM bank-safely: per mt2 region,
                    # C group -> overwrite with r*C -> B accumulates, all
                    # before the next region's start= arms the bank
                    for mt2 in range(2):
                        mt = 2 * h + mt2
                        for kp in range(4):
                            nc.tensor.matmul(
                                cb_ps[h][:, mt2, :], wc[:, kp, mt, :, :],
                                hT[:, kp % 2, :, :],
                                start=(kp == 0), stop=(kp == 3),
                                perf_mode=DR)
                        nc.vector.tensor_mul(cb_ps[h][:, mt2, :],
                                             rs[:, 0, mt2, :],
                                             cb_ps[h][:, mt2, :])
                        for wi in range(2):
                            nc.tensor.matmul(
                                cb_ps[h][:, mt2, :], wb[:, wi, mt, :],
                                hT[:, 1, 1, :],
                                start=False, stop=(wi == 1),
                                skip_group_check=True)
                    if DBG and v == DBGV and h == 0:
                        nc.sync.dma_start(out=d_dbg["dbg_rs"], in_=rs[:])
                    ns = work.tile([128, 2, 256], BF, tag=f"ns{h}", name="ns")
                    n_sb.append(ns)
                    nc.scalar.activation(ns[:], cb_ps[h][:], TANH, scale=SAB)
                    if DBG and v == DBGV and h == 0:
                        nc.sync.dma_start(out=d_dbg["dbg_ns"], in_=ns[:])
                    ds = work.tile([128, 2, 256], BF, tag=f"ds{h}", name="ds")
                    d_sb.append(ds)
                    if v == 0:
                        nc.vector.tensor_scalar_mul(ds[:], ns[:], -1.0)
                    else:
                        nc.vector.scalar_tensor_tensor(
                            out=ds[:], in0=agg_prev[:, 2 * h:2 * h + 2, :],
                            scalar=1.0 / SM, in1=ns[:], op0=MUL, op1=SUB)
                    h2 = work.tile([128, 2, 256], BF, tag=f"h2{h}", name="h2")
                    nc.gpsimd.tensor_mul(h2[:], rs[:, 1, :, :], ds[:])
                    if v < MAX_N - 1:
                        nc.gpsimd.tensor_add(hv8[:, h, :, :], h2[:], ns[:])
                    else:
                        nc.gpsimd.tensor_add(hv16[:, 2 * h:2 * h + 2, :],
                                             h2[:], ns[:])

                # pagg tile allocated here (after the d-STT readers of the
                # previous generation); written after msb[v] below
                agg_ps = None
                if vn < MAX_N:
                    agg_ps = paggp.tile([128, 4, 256], F32, tag="agg",
                                        name="agg")

                if v < MAX_N - 1:
                    # const-1 row (gate/mapper bias rhs); after the adds so
                    # the full-partition writes don't clobber it
                    nc.sync.dma_start(out=hv8[127:, 1, 1, :], in_=d_one[:])

                if DBG and v == DBGV:
                    nc.sync.dma_start(out=d_dbg["dbg_hv"], in_=hv8[:])
                if v == MAX_N - 1:
                    # ---- final FC ----
                    pf = psA.tile([128, 256], F32, tag="ps", name="pf")
                    for kt in range(4):
                        nc.tensor.matmul(pf[:112, :], wf[:, kt, :],
                                         hv16[:, kt, :],
                                         start=(kt == 0), stop=(kt == 3))
                    out_sb = work.tile([128, 256], F32, tag="out_sb",
                                       name="out_sb")
                    nc.scalar.activation(out_sb[:112, :], pf[:112, :], IDENT,
                                         bias=fcb[:112, :])
                    nc.sync.dma_start(out=d_y[:], in_=out_sb[:112, :])
                    break

                # ---- phase B matmuls (gate/mapper) ----
                gm_ps = []
                for h in range(2):
                    p = psA.tile([128, 2, 2, 256], F32, tag="ps", name="gmp")
                    gm_ps.append(p)
                    for gm in range(2):
                        for mt2 in range(2):
                            mt = 2 * h + mt2
                            for kp in range(4):
                                nc.tensor.matmul(
                                    p[:, gm, mt2, :],
                                    wgm[:, kp, gm, mt, :, :],
                                    hv8[:, kp % 2, :, :],
                                    start=(kp == 0), stop=(kp == 3),
                                    perf_mode=DR)

                # ---- gate sigmoid + message build ----
                gmf = []
                for h in range(2):
                    g = work.tile([128, 2, 256], BF, tag=f"g{h}", name="g")
                    nc.scalar.activation(g[:], gm_ps[h][:, 0, :, :], SIG,
                                         scale=1.0 / SG)
                    gf = work.tile([128, 2, 256], BF, tag=f"gmf{h}",
                                   name="gmf")
                    gmf.append(gf)
                    nc.vector.tensor_mul(gf[:], g[:], gm_ps[h][:, 1, :, :])

                if DBG and v == DBGV:
                    nc.sync.dma_start(out=d_dbg["dbg_gm"], in_=gmf[0][:])
                # ---- transpose messages into batch-major msb ----
                ptg = [None, None]
                for bt in range(2):
                    pt = psA.tile([128, 4, 128], BF, tag="cb", name="ptg",
                                  bufs=2)
                    ptg[bt] = pt
                    for ft in range(4):
                        nc.tensor.transpose(
                            pt[:, ft, :],
                            gmf[ft // 2][:, ft % 2,
                                         bt * 128:bt * 128 + 128],
                            id16[:])
                for bt in range(2):
                    nc.vector.tensor_copy(msb[:, v, bt, :], ptg[bt][:])

                if DBG and v == DBGV:
                    nc.sync.dma_start(out=d_dbg["dbg_msb"],
                                      in_=msb[:, 0, :, :])
                # ---- gate/mapper bias rows for the NEXT step ----
                nc.sync.dma_start(
                    out=wgm[127:, 1, :, :, 1, :],
                    in_=d_gbr[:, vn * 2048:vn * 2048 + 1024])
                nc.sync.dma_start(
                    out=wgm[127:, 3, :, :, 1, :],
                    in_=d_gbr[:, vn * 2048 + 1024:(vn + 1) * 2048])

                # ---- aggregation: contiguous group per region, banks
                # visited consecutively (ft-outer) so no accumulate lands
                # after a foreign start= armed its bank ----
                for ft in range(4):
                    for bt in range(2):
                        for idx, u in plan[(vn, bt)]:
                            nc.tensor.matmul(
                                agg_ps[:, ft, bt * 128:bt * 128 + 128],
                                msb[:, u, bt, ft * 128:ft * 128 + 128],
                                msk[:, idx, :],
                                start=(u == 0), stop=(u == vn - 1))

                # ---- stage next GRU rhs (fp8 SBUF) ----
                hT = new_hT(vn)
                nc.vector.tensor_copy(hT[:, 0, :, :], agg_ps[:, 0:2, :])
                nc.vector.tensor_copy(hT[:, 1, 0, :], agg_ps[:, 2, :])
                nc.vector.tensor_copy(hT[:117, 1, 1, :], agg_ps[:117, 3, :])
                if DBG and v == DBGV:
                    dbg_agg_sb = work.tile([128, 4, 256], F32, tag="dbga",
                                           name="dbga")
                    nc.vector.tensor_copy(dbg_agg_sb[:], agg_ps[:])
                    nc.sync.dma_start(out=d_dbg["dbg_agg"],
                                      in_=dbg_agg_sb[:])
                    nc.sync.dma_start(out=d_dbg["dbg_ht1"], in_=hT[:])
                agg_prev = agg_ps

    nc.compile()
    return nc


def _quant8(x):
    import ml_dtypes
    return np.asarray(x, np.float32).astype(ml_dtypes.float8_e4m3)


def _prep_static(w_ih, w_hh, b_ih, b_hh, gate_w, gate_b, map_w,
                 fc1_w, fc1_b, fc2_w, fc2_b):
    import ml_dtypes
    f32 = np.float32
    bf16 = ml_dtypes.bfloat16

    # K-row layout of the GRU rhs h~: 0:501 h (SM-scaled), 501:509 x one-hot,
    # 509 const-1 (GRU bias), 510/511 zero. Weight column scaling: h-cols
    # x SWH (rhs carries SM) -> psum = SWH*SM*(W h); x/bias cols x SWH*SM.
    SXB = SWH * SM

    def gru_w(w_h, w_x, bias):
        W = np.zeros((512, 512), f32)
        W[0:501, 0:501] = w_h.T * SWH
        if w_x is not None:
            W[501:509, 0:501] = w_x.T * SXB
        W[509, 0:501] = bias * SXB
        return W

    bias_rz = (b_ih + b_hh).astype(f32)
    WRZ = np.stack([
        gru_w(w_hh[0:501], w_ih[0:501], bias_rz[0:501]),
        gru_w(w_hh[501:1002], w_ih[501:1002], bias_rz[501:1002])])
    WC = gru_w(w_hh[1002:1503], None, b_hh[1002:1503])
    WB0 = np.zeros((128, 512), f32)
    WB0[117:125, 0:501] = w_ih[1002:1503].T * SXB
    WB0[125, 0:501] = b_ih[1002:1503] * SXB
    WBq = np.asarray(_quant8(WB0), f32)
    WB = np.stack([WBq, WB0 - WBq])      # [hi/lo, 128, 512]

    WGM = np.zeros((2, 512, 512), f32)
    WGM[0, 0:501, 0:501] = gate_w[:, 0:501].T * SG
    WGM[1, 0:501, 0:501] = map_w[:, 0:501].T * SG

    # DoubleRow flats: [kpart, kp, plane, mt, ki, mpart]. For WC and WGM,
    # kp planes 2,3 hold the fp8 quantization residual (contracted against
    # the same rhs k-pairs), recovering near-bf16 weight precision.
    def with_residual(W):
        q = np.asarray(_quant8(W), f32)
        return np.concatenate([q, W - q], axis=-2)   # K axis doubled

    wrz = _quant8(WRZ.reshape(2, 2, 2, 128, 4, 128)
                  .transpose(3, 1, 0, 4, 2, 5).reshape(128, 4096))
    WGMr = with_residual(WGM)                        # [2gm, 1024K, 512]
    wgm = _quant8(WGMr.reshape(2, 4, 2, 128, 4, 128)
                  .transpose(3, 1, 0, 4, 2, 5).reshape(128, 8192))
    WCr = with_residual(WC)                          # [1024K, 512]
    wc = _quant8(WCr.reshape(4, 2, 128, 4, 128)
                 .transpose(2, 0, 3, 1, 4).reshape(128, 4096))
    wb = _quant8(WB.transpose(1, 0, 2).reshape(128, 1024))

    WF = np.zeros((512, 112), f32)
    WF[0:501, 0:56] = fc1_w.T
    WF[0:501, 56:112] = fc2_w.T
    wf = np.ascontiguousarray(
        WF.reshape(4, 128, 112).transpose(1, 0, 2).reshape(128, 448)
    ).astype(bf16)
    fcb = np.zeros((128, 1), f32)
    fcb[0:56, 0] = fc1_b
    fcb[56:112, 0] = fc2_b

    # per-step gate/mapper bias rows with fp8 residual:
    # [1, v, kp-sel(hi/lo), gm, mt, 128] (SG-scaled)
    gbr = np.zeros((MAX_N, 2, 2, 4, 128), f32)
    for v in range(MAX_N):
        gb_full = np.zeros(512, f32)
        mb_full = np.zeros(512, f32)
        gb_full[0:501] = (gate_b + gate_w[:, HS + v]) * SG
        mb_full[0:501] = map_w[:, HS + v] * SG
        for gm, row in ((0, gb_full), (1, mb_full)):
            hi = np.asarray(_quant8(row), f32)
            gbr[v, 0, gm] = hi.reshape(4, 128)
            gbr[v, 1, gm] = (row - hi).reshape(4, 128)
    gbr = _quant8(gbr.reshape(1, MAX_N * 2048))

    one = np.ones((1, 256), f32)
    id16 = np.eye(128, dtype=f32).astype(bf16)
    return dict(wrz=wrz, wc=wc, wb=wb, wgm=wgm, wf=wf, fcb=fcb,
                gbr=gbr, onerow=_quant8(one), id16=id16)


def _prep_core(node_types, adj, core):
    import ml_dtypes
    f32 = np.float32
    plan, NMSK = _mask_plan()
    off = core * BL
    nt = node_types[off:off + BL]          # [256, 16] int32
    ad = adj[off:off + BL].astype(f32)     # [256, 16, 16]

    xh = np.zeros((128, MAX_N, 256), f32)
    for bt in range(2):
        nb = nt[bt * 128:(bt + 1) * 128]   # [128, 16]
        for t in range(NVT):
            xh[117 + t, :, bt * 128:(bt + 1) * 128] = (nb.T == t)
    xh[125, :, :] = 1.0

    msk = np.zeros((128, NMSK, 128), f32)
    di = np.arange(128)
    for (vn, bt), ent in plan.items():
        ab = ad[bt * 128:(bt + 1) * 128]   # [128, 16, 16]
        for idx, u in ent:
            msk[di, idx, di] = ab[:, vn, u]
    return dict(xh=_quant8(xh.reshape(128, MAX_N * 256)),
                msk=msk.reshape(128, NMSK * 128).astype(ml_dtypes.bfloat16))


def kernel(node_types, adj, w_ih, w_hh, b_ih, b_hh, gate_w, gate_b, map_w,
           fc1_w, fc1_b, fc2_w, fc2_b):
    from concourse.bass_utils import run_bass_kernel_spmd

    if "nc" not in _CACHE:
        _CACHE["nc"] = _build_nc()
    nc = _CACHE["nc"]

    node_types = np.asarray(node_types)
    adj = np.asarray(adj, dtype=np.float32)
    static = _prep_static(
        np.asarray(w_ih, np.float32), np.asarray(w_hh, np.float32),
        np.asarray(b_ih, np.float32), np.asarray(b_hh, np.float32),
        np.asarray(gate_w, np.float32), np.asarray(gate_b, np.float32),
        np.asarray(map_w, np.float32),
        np.asarray(fc1_w, np.float32), np.asarray(fc1_b, np.float32),
        np.asarray(fc2_w, np.float32), np.asarray(fc2_b, np.float32))
    in_maps = []
    for c in range(NC_CORES):
        m = dict(static)
        m.update(_prep_core(node_types, adj, c))
        in_maps.append(m)

    res = run_bass_kernel_spmd(nc, in_maps, core_ids=list(range(NC_CORES)))
    ys = [res.results[c]["y"] for c in range(NC_CORES)]   # each [112, 256]
    out = np.concatenate(ys, axis=1).T                     # [2048, 112]
    return np.ascontiguousarray(out.astype(np.float32))


# revision 29
# speedup vs baseline: 1.0076x; 1.0076x over previous
"""DVAE encoder (batched DAG GRU message passing) on 8 trn2 NeuronCores.

Strategy: pure data-parallel over batch (256 graphs/core). Feature-major
compute with fp8-e4m3 DoubleRow matmuls (2 K-tiles per instruction) for the
GRU/gate/mapper contractions. Scale scheme (all recovered by exact pow-2
activation scales): messages carry 16x, GRU weights 8x on h-columns and
128x on x/bias columns, gate/mapper weights 16x. The predecessor
aggregation runs message-stationary on the PE: lhsT = message pairs,
moving = host-prepped diagonal adjacency-mask pairs, so the aggregate is
produced directly in feature-major PSUM (no batch-major detour). Per-step
gate/mapper bias rows, one-hot x rows, and the const-1 row are streamed by
DMA into fp8 weight/rhs rows, off all compute engines. Pointwise work is
split at half-tile granularity across DVE / GpSimd / Scalar so the 16-step
recurrence pipelines.
"""

import numpy as np

B, MAX_N, NVT, HS, NZ = 2048, 16, 8, 501, 56
NC_CORES = 8
BL = B // NC_CORES   # 256 graphs per core
SM = 16.0            # message scale (msb carries SM*gm; agg psum = SM*h_in)
SWH = 8.0            # GRU weight scale on h-columns
SAB = 1.0 / (SWH * SM)   # GRU activation scale
SG = 16.0            # gate/mapper weight scale

_CACHE = {}


def _mask_plan():
    """Aggregation schedule: per (vn, bt) a diagonal mask per predecessor u.
    u < vn-1 are issued early (prefix); u = vn-1 after msb[vn-1] is written.
    Returns ({(vn, bt): [(mask_idx, u)]}, nmasks)."""
    out = {}
    nm = 0
    for vn in range(1, MAX_N):
        for bt in range(2):
            ent = []
            for u in range(vn):
                ent.append((nm, u))
                nm += 1
            out[(vn, bt)] = ent
    return out, nm


def _build_nc():
    import concourse.mybir as mybir
    import concourse.tile as tile
    from concourse import bacc

    F32 = mybir.dt.float32
    BF = mybir.dt.bfloat16
    F8 = mybir.dt.float8e4
    DR = mybir.MatmulPerfMode.DoubleRow
    SIG = mybir.ActivationFunctionType.Sigmoid
    TANH = mybir.ActivationFunctionType.Tanh
    IDENT = mybir.ActivationFunctionType.Identity
    MUL = mybir.AluOpType.mult
    SUB = mybir.AluOpType.subtract
    ADD = mybir.AluOpType.add

    plan, NMSK = _mask_plan()

    nc = bacc.Bacc("TRN2", target_bir_lowering=False, debug=False,
                   num_devices=NC_CORES)

    d_wrz = nc.dram_tensor("wrz", [128, 4096], F8, kind="ExternalInput").ap()
    d_wc = nc.dram_tensor("wc", [128, 4096], F8, kind="ExternalInput").ap()
    d_wb = nc.dram_tensor("wb", [128, 1024], F8, kind="ExternalInput").ap()
    d_wgm = nc.dram_tensor("wgm", [128, 8192], F8, kind="ExternalInput").ap()
    d_wf = nc.dram_tensor("wf", [128, 448], BF, kind="ExternalInput").ap()
    d_fcb = nc.dram_tensor("fcb", [128, 1], F32, kind="ExternalInput").ap()
    d_xh = nc.dram_tensor("xh", [128, MAX_N * 256], F8,
                          kind="ExternalInput").ap()
    d_gbr = nc.dram_tensor("gbr", [1, MAX_N * 2048], F8,
                           kind="ExternalInput").ap()
    d_one = nc.dram_tensor("onerow", [1, 256], F8, kind="ExternalInput").ap()
    d_msk = nc.dram_tensor("msk", [128, NMSK * 128], BF,
                           kind="ExternalInput").ap()
    d_id16 = nc.dram_tensor("id16", [128, 128], BF,
                            kind="ExternalInput").ap()
    d_y = nc.dram_tensor("y", [112, BL], F32, kind="ExternalOutput").ap()
    import os
    DBG = os.environ.get("KDBG", "0") == "1"
    DBGV = int(os.environ.get("KDBG_V", "0"))
    if DBG:
        d_dbg = {}
        for nm, shp, dt in (("dbg_ht0", [128, 1024], F8),
                            ("dbg_rs", [128, 1024], BF),
                            ("dbg_ns", [128, 512], BF),
                            ("dbg_hv", [128, 1024], F8),
                            ("dbg_gm", [128, 512], BF),
                            ("dbg_msb", [128, 1024], BF),
                            ("dbg_agg", [128, 1024], F32),
                            ("dbg_ht1", [128, 1024], F8)):
            d_dbg[nm] = nc.dram_tensor(nm, shp, dt,
                                       kind="ExternalOutput").ap()

    with tile.TileContext(nc) as tc:
        with tc.tile_pool(name="wts", bufs=1) as wts, \
             tc.tile_pool(name="state", bufs=1) as state, \
             tc.tile_pool(name="hio", bufs=2) as hio, \
             tc.tile_pool(name="work", bufs=2) as work, \
             tc.tile_pool(name="psA", bufs=2, space="PSUM") as psA, \
             tc.tile_pool(name="pagg", bufs=1, space="PSUM") as paggp:

            wrz = wts.tile([128, 2, 2, 4, 2, 128], F8, tag="wrz", name="wrz")
            wc = wts.tile([128, 4, 4, 2, 128], F8, tag="wc", name="wc")
            wb = wts.tile([128, 2, 4, 128], F8, tag="wb", name="wb")
            wf = wts.tile([128, 4, 112], BF, tag="wf", name="wf")
            fcb = wts.tile([128, 1], F32, tag="fcb", name="fcb")
            id16 = wts.tile([128, 128], BF, tag="id16", name="id16")
            msk = wts.tile([128, NMSK, 128], BF, tag="msk", name="msk")
            for t, d in ((wrz, d_wrz), (wc, d_wc), (wb, d_wb), (wf, d_wf),
                         (fcb, d_fcb), (id16, d_id16), (msk, d_msk)):
                nc.sync.dma_start(out=t[:], in_=d[:])

            # mutable weight tile (per-step bias rows DMA'd into row 127)
            wgm = state.tile([128, 4, 2, 4, 2, 128], F8, tag="wgm",
                             name="wgm")
            nc.sync.dma_start(out=wgm[:], in_=d_wgm[:])
            # step-0 gate/mapper bias rows (later steps stream inside loop)
            nc.sync.dma_start(out=wgm[127:, 1, :, :, 1, :],
                              in_=d_gbr[:, 0:1024])
            nc.sync.dma_start(out=wgm[127:, 3, :, :, 1, :],
                              in_=d_gbr[:, 1024:2048])
            # messages, batch-major: [128b, u, bt, 512f] (SM-scaled fp8)
            msb = state.tile([128, MAX_N, 2, 512], BF, tag="msb", name="msb")

            def new_hT(v):
                # rhs tile [128, kp, ki, 256] for the GRU contraction of
                # step v; x one-hot + GRU bias + zero rows (117:128 of
                # k-tile 3) are DMA'd from the host-prepped table.
                hT = hio.tile([128, 2, 2, 256], F8, tag="hT", name="hT")
                nc.sync.dma_start(out=hT[117:128, 1, 1, :],
                                  in_=d_xh[117:128, v * 256:(v + 1) * 256])
                return hT

            hT = hio.tile([128, 2, 2, 256], F8, tag="hT", name="hT")
            nc.vector.memset(hT[:], 0.0)
            nc.sync.dma_start(out=hT[117:128, 1, 1, :],
                              in_=d_xh[117:128, 0:256])

            for v in range(MAX_N):
                vn = v + 1
                if DBG and v == DBGV:
                    nc.sync.dma_start(out=d_dbg["dbg_ht0"], in_=hT[:])
                # ---- phase A matmuls ----
                rz_ps, cb_ps = [], []
                for h in range(2):
                    p = psA.tile([128, 2, 2, 256], F32, tag="ps", name="rzp")
                    rz_ps.append(p)
                    for rz in range(2):
                        for mt2 in range(2):
                            mt = 2 * h + mt2
                            for kp in range(2):
                                nc.tensor.matmul(
                                    p[:, rz, mt2, :],
                                    wrz[:, kp, rz, mt, :, :],
                                    hT[:, kp, :, :],
                                    start=(kp == 0), stop=(kp == 1),
                                    perf_mode=DR)
                for h in range(2):
                    p = psA.tile([128, 2, 256], F32, tag="cb", name="cbp",
                                 bufs=2)
                    cb_ps.append(p)
                    for mt2 in range(2):
                        mt = 2 * h + mt2
                        for kp in range(4):
                            nc.tensor.matmul(
                                p[:, mt2, :], wc[:, kp, mt, :, :],
                                hT[:, kp % 2, :, :],
                                start=(kp == 0), stop=(kp == 3),
                                perf_mode=DR)

                # pagg tile allocated here (after the d-STT readers of the
                # previous generation); written after msb[v] below
                agg_ps = None
                if vn < MAX_N:
                    agg_ps = paggp.tile([128, 4, 256], F32, tag="agg",
                                        name="agg")

                # ---- GRU pointwise ----
                rzs, n_sb, d_sb = [], [], []
                if v < MAX_N - 1:
                    hv8 = work.tile([128, 2, 2, 256], F8, tag="hv8",
                                    name="hv8")
                else:
                    hv16 = work.tile([128, 4, 256], BF, tag="hv16",
                                     name="hv16")
                for h in range(2):
                    rs = work.tile([128, 2, 2, 256], BF, tag=f"rzs{h}",
                                   name="rzs")
                    rzs.append(rs)
                    nc.scalar.activation(rs[:], rz_ps[h][:], SIG, scale=SAB)
                    # pre_n built in PSUM bank-safely: per mt2 region,
                    # C group -> overwrite with r*C -> B accumulates, all
                    # before the next region's start= arms the bank
                    for mt2 in range(2):
                        mt = 2 * h + mt2
                        for kp in range(4):
                            nc.tensor.matmul(
                                cb_ps[h][:, mt2, :], wc[:, kp, mt, :, :],
                                hT[:, kp % 2, :, :],
                                start=(kp == 0), stop=(kp == 3),
                                perf_mode=DR)
                        nc.vector.tensor_mul(cb_ps[h][:, mt2, :],
                                             rs[:, 0, mt2, :],
                                             cb_ps[h][:, mt2, :])
                        for wi in range(2):
                            nc.tensor.matmul(
                                cb_ps[h][:, mt2, :], wb[:, wi, mt, :],
                                hT[:, 1, 1, :],
                                start=False, stop=(wi == 1),
                                skip_group_check=True)
                    if DBG and v == DBGV and h == 0:
                        nc.sync.dma_start(out=d_dbg["dbg_rs"], in_=rs[:])
                    ns = work.tile([128, 2, 256], BF, tag=f"ns{h}", name="ns")
                    n_sb.append(ns)
                    nc.scalar.activation(ns[:], cb_ps[h][:], TANH, scale=SAB)
                    if DBG and v == DBGV and h == 0:
                        nc.sync.dma_start(out=d_dbg["dbg_ns"], in_=ns[:])
                    ds = work.tile([128, 2, 256], BF, tag=f"ds{h}", name="ds")
                    d_sb.append(ds)
                    if v == 0:
                        nc.vector.tensor_scalar_mul(ds[:], ns[:], -1.0)
                    else:
                        nc.vector.scalar_tensor_tensor(
                            out=ds[:], in0=agg_prev[:, 2 * h:2 * h + 2, :],
                            scalar=1.0 / SM, in1=ns[:], op0=MUL, op1=SUB)
                    h2 = work.tile([128, 2, 256], BF, tag=f"h2{h}", name="h2")
                    nc.gpsimd.tensor_mul(h2[:], rs[:, 1, :, :], ds[:])
                    if v < MAX_N - 1:
                        nc.gpsimd.tensor_add(hv8[:, h, :, :], h2[:], ns[:])
                    else:
                        nc.gpsimd.tensor_add(hv16[:, 2 * h:2 * h + 2, :],
                                             h2[:], ns[:])


                if v < MAX_N - 1:
                    # const-1 row (gate/mapper bias rhs); after the adds so
                    # the full-partition writes don't clobber it
                    nc.sync.dma_start(out=hv8[127:, 1, 1, :], in_=d_one[:])

                if DBG and v == DBGV:
                    nc.sync.dma_start(out=d_dbg["dbg_hv"], in_=hv8[:])
                if v == MAX_N - 1:
                    # ---- final FC ----
                    pf = psA.tile([128, 256], F32, tag="ps", name="pf")
                    for kt in range(4):
                        nc.tensor.matmul(pf[:112, :], wf[:, kt, :],
                                         hv16[:, kt, :],
                                         start=(kt == 0), stop=(kt == 3))
                    out_sb = work.tile([128, 256], F32, tag="out_sb",
                                       name="out_sb")
                    nc.scalar.activation(out_sb[:112, :], pf[:112, :], IDENT,
                                         bias=fcb[:112, :])
                    nc.sync.dma_start(out=d_y[:], in_=out_sb[:112, :])
                    break

                # ---- phase B matmuls (gate/mapper) ----
                gm_ps = []
                for h in range(2):
                    p = psA.tile([128, 2, 2, 256], F32, tag="ps", name="gmp")
                    gm_ps.append(p)
                    for gm in range(2):
                        for mt2 in range(2):
                            mt = 2 * h + mt2
                            for kp in range(4):
                                nc.tensor.matmul(
                                    p[:, gm, mt2, :],
                                    wgm[:, kp, gm, mt, :, :],
                                    hv8[:, kp % 2, :, :],
                                    start=(kp == 0), stop=(kp == 3),
                                    perf_mode=DR)

                # ---- gate sigmoid + message build ----
                gmf = []
                for h in range(2):
                    g = work.tile([128, 2, 256], BF, tag=f"g{h}", name="g")
                    nc.scalar.activation(g[:], gm_ps[h][:, 0, :, :], SIG,
                                         scale=1.0 / SG)
                    gf = work.tile([128, 2, 256], BF, tag=f"gmf{h}",
                                   name="gmf")
                    gmf.append(gf)
                    nc.vector.tensor_mul(gf[:], g[:], gm_ps[h][:, 1, :, :])

                if DBG and v == DBGV:
                    nc.sync.dma_start(out=d_dbg["dbg_gm"], in_=gmf[0][:])
                # ---- transpose messages into batch-major msb ----
                ptg = [None, None]
                for bt in range(2):
                    pt = psA.tile([128, 4, 128], BF, tag="cb", name="ptg",
                                  bufs=2)
                    ptg[bt] = pt
                    for ft in range(4):
                        nc.tensor.transpose(
                            pt[:, ft, :],
                            gmf[ft // 2][:, ft % 2,
                                         bt * 128:bt * 128 + 128],
                            id16[:])
                for bt in range(2):
                    nc.vector.tensor_copy(msb[:, v, bt, :], ptg[bt][:])

                if DBG and v == DBGV:
                    nc.sync.dma_start(out=d_dbg["dbg_msb"],
                                      in_=msb[:, 0, :, :])
                # ---- gate/mapper bias rows for the NEXT step ----
                nc.sync.dma_start(
                    out=wgm[127:, 1, :, :, 1, :],
                    in_=d_gbr[:, vn * 2048:vn * 2048 + 1024])
                nc.sync.dma_start(
                    out=wgm[127:, 3, :, :, 1, :],
                    in_=d_gbr[:, vn * 2048 + 1024:(vn + 1) * 2048])

                # ---- aggregation: contiguous group per region, banks
                # visited consecutively so no accumulate lands after a
                # foreign start= armed its bank ----
                for ft in range(4):
                    for bt in range(2):
                        for idx, u in plan[(vn, bt)]:
                            nc.tensor.matmul(
                                agg_ps[:, ft, bt * 128:bt * 128 + 128],
                                msb[:, u, bt, ft * 128:ft * 128 + 128],
                                msk[:, idx, :],
                                start=(u == 0), stop=(u == vn - 1))

                # ---- stage next GRU rhs (fp8 SBUF) ----
                hT = new_hT(vn)
                nc.vector.tensor_copy(hT[:, 0, :, :], agg_ps[:, 0:2, :])
                nc.vector.tensor_copy(hT[:, 1, 0, :], agg_ps[:, 2, :])
                nc.vector.tensor_copy(hT[:117, 1, 1, :], agg_ps[:117, 3, :])
                if DBG and v == DBGV:
                    nc.sync.dma_start(out=d_dbg["dbg_ht1"], in_=hT[:])
                agg_prev = agg_ps

    nc.compile()
    return nc


def _quant8(x):
    import ml_dtypes
    return np.asarray(x, np.float32).astype(ml_dtypes.float8_e4m3)


def _prep_static(w_ih, w_hh, b_ih, b_hh, gate_w, gate_b, map_w,
                 fc1_w, fc1_b, fc2_w, fc2_b):
    import ml_dtypes
    f32 = np.float32
    bf16 = ml_dtypes.bfloat16

    # K-row layout of the GRU rhs h~: 0:501 h (SM-scaled), 501:509 x one-hot,
    # 509 const-1 (GRU bias), 510/511 zero. Weight column scaling: h-cols
    # x SWH (rhs carries SM) -> psum = SWH*SM*(W h); x/bias cols x SWH*SM.
    SXB = SWH * SM

    def gru_w(w_h, w_x, bias):
        W = np.zeros((512, 512), f32)
        W[0:501, 0:501] = w_h.T * SWH
        if w_x is not None:
            W[501:509, 0:501] = w_x.T * SXB
        W[509, 0:501] = bias * SXB
        return W

    bias_rz = (b_ih + b_hh).astype(f32)
    WRZ = np.stack([
        gru_w(w_hh[0:501], w_ih[0:501], bias_rz[0:501]),
        gru_w(w_hh[501:1002], w_ih[501:1002], bias_rz[501:1002])])
    WC = gru_w(w_hh[1002:1503], None, b_hh[1002:1503])
    WB0 = np.zeros((128, 512), f32)
    WB0[117:125, 0:501] = w_ih[1002:1503].T * SXB
    WB0[125, 0:501] = b_ih[1002:1503] * SXB
    WBq = np.asarray(_quant8(WB0), f32)
    WB = np.stack([WBq, WB0 - WBq])      # [hi/lo, 128, 512]

    WGM = np.zeros((2, 512, 512), f32)
    WGM[0, 0:501, 0:501] = gate_w[:, 0:501].T * SG
    WGM[1, 0:501, 0:501] = map_w[:, 0:501].T * SG

    # DoubleRow flats: [kpart, kp, plane, mt, ki, mpart]. For WC and WGM,
    # kp planes 2,3 hold the fp8 quantization residual (contracted against
    # the same rhs k-pairs), recovering near-bf16 weight precision.
    def with_residual(W):
        q = np.asarray(_quant8(W), f32)
        return np.concatenate([q, W - q], axis=-2)   # K axis doubled

    wrz = _quant8(WRZ.reshape(2, 2, 2, 128, 4, 128)
                  .transpose(3, 1, 0, 4, 2, 5).reshape(128, 4096))
    WGMr = with_residual(WGM)                        # [2gm, 1024K, 512]
    wgm = _quant8(WGMr.reshape(2, 4, 2, 128, 4, 128)
                  .transpose(3, 1, 0, 4, 2, 5).reshape(128, 8192))
    WCr = with_residual(WC)                          # [1024K, 512]
    wc = _quant8(WCr.reshape(4, 2, 128, 4, 128)
                 .transpose(2, 0, 3, 1, 4).reshape(128, 4096))
    wb = _quant8(WB.transpose(1, 0, 2).reshape(128, 1024))

    WF = np.zeros((512, 112), f32)
    WF[0:501, 0:56] = fc1_w.T
    WF[0:501, 56:112] = fc2_w.T
    wf = np.ascontiguousarray(
        WF.reshape(4, 128, 112).transpose(1, 0, 2).reshape(128, 448)
    ).astype(bf16)
    fcb = np.zeros((128, 1), f32)
    fcb[0:56, 0] = fc1_b
    fcb[56:112, 0] = fc2_b

    # per-step gate/mapper bias rows with fp8 residual:
    # [1, v, kp-sel(hi/lo), gm, mt, 128] (SG-scaled)
    gbr = np.zeros((MAX_N, 2, 2, 4, 128), f32)
    for v in range(MAX_N):
        gb_full = np.zeros(512, f32)
        mb_full = np.zeros(512, f32)
        gb_full[0:501] = (gate_b + gate_w[:, HS + v]) * SG
        mb_full[0:501] = map_w[:, HS + v] * SG
        for gm, row in ((0, gb_full), (1, mb_full)):
            hi = np.asarray(_quant8(row), f32)
            gbr[v, 0, gm] = hi.reshape(4, 128)
            gbr[v, 1, gm] = (row - hi).reshape(4, 128)
    gbr = _quant8(gbr.reshape(1, MAX_N * 2048))

    one = np.ones((1, 256), f32)
    id16 = np.eye(128, dtype=f32).astype(bf16)
    return dict(wrz=wrz, wc=wc, wb=wb, wgm=wgm, wf=wf, fcb=fcb,
                gbr=gbr, onerow=_quant8(one), id16=id16)


def _prep_core(node_types, adj, core):
    import ml_dtypes
    f32 = np.float32
    plan, NMSK = _mask_plan()
    off = core * BL
    nt = node_types[off:off + BL]          # [256, 16] int32
    ad = adj[off:off + BL].astype(f32)     # [256, 16, 16]

    xh = np.zeros((128, MAX_N, 256), f32)
    for bt in range(2):
        nb = nt[bt * 128:(bt + 1) * 128]   # [128, 16]
        for t in range(NVT):
            xh[117 + t, :, bt * 128:(bt + 1) * 128] = (nb.T == t)
    xh[125, :, :] = 1.0

    msk = np.zeros((128, NMSK, 128), f32)
    di = np.arange(128)
    for (vn, bt), ent in plan.items():
        ab = ad[bt * 128:(bt + 1) * 128]   # [128, 16, 16]
        for idx, u in ent:
            msk[di, idx, di] = ab[:, vn, u]
    return dict(xh=_quant8(xh.reshape(128, MAX_N * 256)),
                msk=msk.reshape(128, NMSK * 128).astype(ml_dtypes.bfloat16))


def kernel(node_types, adj, w_ih, w_hh, b_ih, b_hh, gate_w, gate_b, map_w,
           fc1_w, fc1_b, fc2_w, fc2_b):
    from concourse.bass_utils import run_bass_kernel_spmd

    if "nc" not in _CACHE:
        _CACHE["nc"] = _build_nc()
    nc = _CACHE["nc"]

    node_types = np.asarray(node_types)
    adj = np.asarray(adj, dtype=np.float32)
    static = _prep_static(
        np.asarray(w_ih, np.float32), np.asarray(w_hh, np.float32),
        np.asarray(b_ih, np.float32), np.asarray(b_hh, np.float32),
        np.asarray(gate_w, np.float32), np.asarray(gate_b, np.float32),
        np.asarray(map_w, np.float32),
        np.asarray(fc1_w, np.float32), np.asarray(fc1_b, np.float32),
        np.asarray(fc2_w, np.float32), np.asarray(fc2_b, np.float32))
    in_maps = []
    for c in range(NC_CORES):
        m = dict(static)
        m.update(_prep_core(node_types, adj, c))
        in_maps.append(m)

    res = run_bass_kernel_spmd(nc, in_maps, core_ids=list(range(NC_CORES)))
    ys = [res.results[c]["y"] for c in range(NC_CORES)]   # each [112, 256]
    out = np.concatenate(ys, axis=1).T                     # [2048, 112]
    return np.ascontiguousarray(out.astype(np.float32))


# revision 30
# speedup vs baseline: 1.0971x; 1.0889x over previous
"""DVAE encoder (batched DAG GRU message passing) on 8 trn2 NeuronCores.

Strategy: pure data-parallel over batch (256 graphs/core). Feature-major
compute with fp8-e4m3 DoubleRow matmuls (2 K-tiles per instruction) for the
GRU/gate/mapper contractions. Scale scheme (all recovered by exact pow-2
activation scales): messages carry 16x, GRU weights 8x on h-columns and
128x on x/bias columns, gate/mapper weights 16x. The predecessor
aggregation runs message-stationary on the PE: lhsT = message pairs,
moving = host-prepped diagonal adjacency-mask pairs, so the aggregate is
produced directly in feature-major PSUM (no batch-major detour). Per-step
gate/mapper bias rows, one-hot x rows, and the const-1 row are streamed by
DMA into fp8 weight/rhs rows, off all compute engines. Pointwise work is
split at half-tile granularity across DVE / GpSimd / Scalar so the 16-step
recurrence pipelines.
"""

import numpy as np

B, MAX_N, NVT, HS, NZ = 2048, 16, 8, 501, 56
NC_CORES = 8
BL = B // NC_CORES   # 256 graphs per core
SM = 16.0            # message scale (msb carries SM*gm; agg psum = SM*h_in)
SWH = 8.0            # GRU weight scale on h-columns
SAB = 1.0 / (SWH * SM)   # GRU activation scale
SG = 16.0            # gate/mapper weight scale

_CACHE = {}


def _mask_plan():
    """Aggregation schedule: per (vn, bt) a diagonal mask per predecessor u.
    u < vn-1 are issued early (prefix); u = vn-1 after msb[vn-1] is written.
    Returns ({(vn, bt): [(mask_idx, u)]}, nmasks)."""
    out = {}
    nm = 0
    for vn in range(1, MAX_N):
        for bt in range(2):
            ent = []
            for u in range(vn):
                ent.append((nm, u))
                nm += 1
            out[(vn, bt)] = ent
    return out, nm


def _build_nc():
    import concourse.mybir as mybir
    import concourse.tile as tile
    from concourse import bacc

    F32 = mybir.dt.float32
    BF = mybir.dt.bfloat16
    F8 = mybir.dt.float8e4
    DR = mybir.MatmulPerfMode.DoubleRow
    SIG = mybir.ActivationFunctionType.Sigmoid
    TANH = mybir.ActivationFunctionType.Tanh
    IDENT = mybir.ActivationFunctionType.Identity
    MUL = mybir.AluOpType.mult
    SUB = mybir.AluOpType.subtract
    ADD = mybir.AluOpType.add

    plan, NMSK = _mask_plan()

    nc = bacc.Bacc("TRN2", target_bir_lowering=False, debug=False,
                   num_devices=NC_CORES)

    d_wrz = nc.dram_tensor("wrz", [128, 4096], F8, kind="ExternalInput").ap()
    d_wc = nc.dram_tensor("wc", [128, 4096], F8, kind="ExternalInput").ap()
    d_wb = nc.dram_tensor("wb", [128, 1024], F8, kind="ExternalInput").ap()
    d_wgm = nc.dram_tensor("wgm", [128, 8192], F8, kind="ExternalInput").ap()
    d_wf = nc.dram_tensor("wf", [128, 448], BF, kind="ExternalInput").ap()
    d_fcb = nc.dram_tensor("fcb", [128, 1], F32, kind="ExternalInput").ap()
    d_xh = nc.dram_tensor("xh", [128, MAX_N * 256], F8,
                          kind="ExternalInput").ap()
    d_gbr = nc.dram_tensor("gbr", [1, MAX_N * 2048], F8,
                           kind="ExternalInput").ap()
    d_one = nc.dram_tensor("onerow", [1, 256], F8, kind="ExternalInput").ap()
    d_msk = nc.dram_tensor("msk", [128, NMSK * 128], BF,
                           kind="ExternalInput").ap()
    d_id16 = nc.dram_tensor("id16", [128, 128], BF,
                            kind="ExternalInput").ap()
    d_y = nc.dram_tensor("y", [112, BL], F32, kind="ExternalOutput").ap()
    import os
    DBG = os.environ.get("KDBG", "0") == "1"
    DBGV = int(os.environ.get("KDBG_V", "0"))
    if DBG:
        d_dbg = {}
        for nm, shp, dt in (("dbg_ht0", [128, 1024], F8),
                            ("dbg_rs", [128, 1024], BF),
                            ("dbg_ns", [128, 512], BF),
                            ("dbg_hv", [128, 1024], F8),
                            ("dbg_gm", [128, 512], BF),
                            ("dbg_msb", [128, 1024], BF),
                            ("dbg_agg", [128, 1024], F32),
                            ("dbg_ht1", [128, 1024], F8)):
            d_dbg[nm] = nc.dram_tensor(nm, shp, dt,
                                       kind="ExternalOutput").ap()

    with tile.TileContext(nc) as tc:
        with tc.tile_pool(name="wts", bufs=1) as wts, \
             tc.tile_pool(name="state", bufs=1) as state, \
             tc.tile_pool(name="hio", bufs=2) as hio, \
             tc.tile_pool(name="work", bufs=2) as work, \
             tc.tile_pool(name="psA", bufs=2, space="PSUM") as psA, \
             tc.tile_pool(name="pagg", bufs=1, space="PSUM") as paggp:

            wrz = wts.tile([128, 2, 2, 4, 2, 128], F8, tag="wrz", name="wrz")
            wc = wts.tile([128, 4, 4, 2, 128], F8, tag="wc", name="wc")
            wb = wts.tile([128, 2, 4, 128], F8, tag="wb", name="wb")
            wf = wts.tile([128, 4, 112], BF, tag="wf", name="wf")
            fcb = wts.tile([128, 1], F32, tag="fcb", name="fcb")
            id16 = wts.tile([128, 128], BF, tag="id16", name="id16")
            msk = wts.tile([128, NMSK, 128], BF, tag="msk", name="msk")
            for t, d in ((wrz, d_wrz), (wc, d_wc), (wb, d_wb), (wf, d_wf),
                         (fcb, d_fcb), (id16, d_id16), (msk, d_msk)):
                nc.sync.dma_start(out=t[:], in_=d[:])

            # mutable weight tile (per-step bias rows DMA'd into row 127)
            wgm = state.tile([128, 4, 2, 4, 2, 128], F8, tag="wgm",
                             name="wgm")
            nc.sync.dma_start(out=wgm[:], in_=d_wgm[:])
            # step-0 gate/mapper bias rows (later steps stream inside loop)
            nc.sync.dma_start(out=wgm[127:, 1, :, :, 1, :],
                              in_=d_gbr[:, 0:1024])
            nc.sync.dma_start(out=wgm[127:, 3, :, :, 1, :],
                              in_=d_gbr[:, 1024:2048])
            # messages, batch-major: [128b, u, bt, 512f] (SM-scaled fp8)
            msb = state.tile([128, MAX_N, 2, 512], BF, tag="msb", name="msb")

            def new_hT(v):
                # rhs tile [128, kp, ki, 256] for the GRU contraction of
                # step v; x one-hot + GRU bias + zero rows (117:128 of
                # k-tile 3) are DMA'd from the host-prepped table.
                hT = hio.tile([128, 2, 2, 256], F8, tag="hT", name="hT")
                nc.sync.dma_start(out=hT[117:128, 1, 1, :],
                                  in_=d_xh[117:128, v * 256:(v + 1) * 256])
                return hT

            hT = hio.tile([128, 2, 2, 256], F8, tag="hT", name="hT")
            nc.vector.memset(hT[:], 0.0)
            nc.sync.dma_start(out=hT[117:128, 1, 1, :],
                              in_=d_xh[117:128, 0:256])

            for v in range(MAX_N):
                vn = v + 1
                if DBG and v == DBGV:
                    nc.sync.dma_start(out=d_dbg["dbg_ht0"], in_=hT[:])
                # ---- phase A matmuls ----
                rz_ps, cb_ps = [], []
                for h in range(2):
                    p = psA.tile([128, 2, 2, 256], F32, tag="ps", name="rzp")
                    rz_ps.append(p)
                    for rz in range(2):
                        for mt2 in range(2):
                            mt = 2 * h + mt2
                            for kp in range(2):
                                nc.tensor.matmul(
                                    p[:, rz, mt2, :],
                                    wrz[:, kp, rz, mt, :, :],
                                    hT[:, kp, :, :],
                                    start=(kp == 0), stop=(kp == 1),
                                    perf_mode=DR)
                for h in range(2):
                    p = psA.tile([128, 2, 256], F32, tag="cb", name="cbp",
                                 bufs=2)
                    cb_ps.append(p)
                    for mt2 in range(2):
                        mt = 2 * h + mt2
                        for kp in range(4):
                            nc.tensor.matmul(
                                p[:, mt2, :], wc[:, kp, mt, :, :],
                                hT[:, kp % 2, :, :],
                                start=(kp == 0), stop=(kp == 3),
                                perf_mode=DR)

                # pagg tile allocated here (after the d-STT readers of the
                # previous generation); written after msb[v] below
                agg_ps = None
                if vn < MAX_N:
                    agg_ps = paggp.tile([128, 4, 256], F32, tag="agg",
                                        name="agg")

                # ---- GRU pointwise ----
                rzs, n_sb, d_sb = [], [], []
                if v < MAX_N - 1:
                    hv8 = work.tile([128, 2, 2, 256], F8, tag="hv8",
                                    name="hv8")
                else:
                    hv16 = work.tile([128, 4, 256], BF, tag="hv16",
                                     name="hv16")
                for h in range(2):
                    rs = work.tile([128, 2, 2, 256], BF, tag=f"rzs{h}",
                                   name="rzs")
                    rzs.append(rs)
                # r first (feeds the r*C mul immediately), z later (only
                # needed at h2)
                for h in range(2):
                    nc.scalar.activation(rzs[h][:, 0, :, :],
                                         rz_ps[h][:, 0, :, :], SIG,
                                         scale=SAB)
                for h in range(2):
                    nc.scalar.activation(rzs[h][:, 1, :, :],
                                         rz_ps[h][:, 1, :, :], SIG,
                                         scale=SAB)
                # pre_n in PSUM bank-safely, halves interleaved so the PE
                # fills the sigma/mul wait of one half with the other
                # half's C group (different bank, so no pending-zero risk)
                for mt2 in range(2):
                    for h in range(2):
                        mt = 2 * h + mt2
                        for kp in range(4):
                            nc.tensor.matmul(
                                cb_ps[h][:, mt2, :], wc[:, kp, mt, :, :],
                                hT[:, kp % 2, :, :],
                                start=(kp == 0), stop=(kp == 3),
                                perf_mode=DR)
                    for h in range(2):
                        nc.vector.tensor_mul(cb_ps[h][:, mt2, :],
                                             rzs[h][:, 0, mt2, :],
                                             cb_ps[h][:, mt2, :])
                    for h in range(2):
                        mt = 2 * h + mt2
                        for wi in range(2):
                            nc.tensor.matmul(
                                cb_ps[h][:, mt2, :], wb[:, wi, mt, :],
                                hT[:, 1, 1, :],
                                start=False, stop=(wi == 1),
                                skip_group_check=True)
                if DBG and v == DBGV:
                    nc.sync.dma_start(out=d_dbg["dbg_rs"], in_=rzs[0][:])
                for h in range(2):
                    ns = work.tile([128, 2, 256], BF, tag=f"ns{h}", name="ns")
                    n_sb.append(ns)
                    nc.scalar.activation(ns[:], cb_ps[h][:], TANH, scale=SAB)
                if DBG and v == DBGV:
                    nc.sync.dma_start(out=d_dbg["dbg_ns"], in_=n_sb[0][:])
                for h in range(2):
                    ns = n_sb[h]
                    ds = work.tile([128, 2, 256], BF, tag=f"ds{h}", name="ds")
                    d_sb.append(ds)
                    if v == 0:
                        nc.vector.tensor_scalar_mul(ds[:], ns[:], -1.0)
                    else:
                        nc.vector.scalar_tensor_tensor(
                            out=ds[:], in0=agg_prev[:, 2 * h:2 * h + 2, :],
                            scalar=1.0 / SM, in1=ns[:], op0=MUL, op1=SUB)
                    h2 = work.tile([128, 2, 256], BF, tag=f"h2{h}", name="h2")
                    nc.vector.tensor_mul(h2[:], rzs[h][:, 1, :, :], ds[:])
                    if v < MAX_N - 1:
                        nc.vector.tensor_add(hv8[:, h, :, :], h2[:], ns[:])
                    else:
                        nc.vector.tensor_add(hv16[:, 2 * h:2 * h + 2, :],
                                             h2[:], ns[:])


                if v < MAX_N - 1:
                    # const-1 row (gate/mapper bias rhs); after the adds so
                    # the full-partition writes don't clobber it
                    nc.sync.dma_start(out=hv8[127:, 1, 1, :], in_=d_one[:])

                if DBG and v == DBGV:
                    nc.sync.dma_start(out=d_dbg["dbg_hv"], in_=hv8[:])
                if v == MAX_N - 1:
                    # ---- final FC ----
                    pf = psA.tile([128, 256], F32, tag="ps", name="pf")
                    for kt in range(4):
                        nc.tensor.matmul(pf[:112, :], wf[:, kt, :],
                                         hv16[:, kt, :],
                                         start=(kt == 0), stop=(kt == 3))
                    out_sb = work.tile([128, 256], F32, tag="out_sb",
                                       name="out_sb")
                    nc.scalar.activation(out_sb[:112, :], pf[:112, :], IDENT,
                                         bias=fcb[:112, :])
                    nc.sync.dma_start(out=d_y[:], in_=out_sb[:112, :])
                    break

                # ---- phase B matmuls (gate/mapper) ----
                gm_ps = []
                for h in range(2):
                    p = psA.tile([128, 2, 2, 256], F32, tag="ps", name="gmp")
                    gm_ps.append(p)
                    for gm in range(2):
                        for mt2 in range(2):
                            mt = 2 * h + mt2
                            for kp in range(4):
                                nc.tensor.matmul(
                                    p[:, gm, mt2, :],
                                    wgm[:, kp, gm, mt, :, :],
                                    hv8[:, kp % 2, :, :],
                                    start=(kp == 0), stop=(kp == 3),
                                    perf_mode=DR)

                # ---- gate sigmoid + message build ----
                gmf = []
                for h in range(2):
                    g = work.tile([128, 2, 256], BF, tag=f"g{h}", name="g")
                    nc.scalar.activation(g[:], gm_ps[h][:, 0, :, :], SIG,
                                         scale=1.0 / SG)
                    gf = work.tile([128, 2, 256], BF, tag=f"gmf{h}",
                                   name="gmf")
                    gmf.append(gf)
                    nc.vector.tensor_mul(gf[:], g[:], gm_ps[h][:, 1, :, :])

                if DBG and v == DBGV:
                    nc.sync.dma_start(out=d_dbg["dbg_gm"], in_=gmf[0][:])
                # ---- transpose messages into batch-major msb ----
                ptg = [None, None]
                for bt in range(2):
                    pt = psA.tile([128, 4, 128], BF, tag="cb", name="ptg",
                                  bufs=2)
                    ptg[bt] = pt
                    for ft in range(4):
                        nc.tensor.transpose(
                            pt[:, ft, :],
                            gmf[ft // 2][:, ft % 2,
                                         bt * 128:bt * 128 + 128],
                            id16[:])
                for bt in range(2):
                    nc.vector.tensor_copy(msb[:, v, bt, :], ptg[bt][:])

                if DBG and v == DBGV:
                    nc.sync.dma_start(out=d_dbg["dbg_msb"],
                                      in_=msb[:, 0, :, :])
                # ---- gate/mapper bias rows for the NEXT step ----
                nc.sync.dma_start(
                    out=wgm[127:, 1, :, :, 1, :],
                    in_=d_gbr[:, vn * 2048:vn * 2048 + 1024])
                nc.sync.dma_start(
                    out=wgm[127:, 3, :, :, 1, :],
                    in_=d_gbr[:, vn * 2048 + 1024:(vn + 1) * 2048])

                # ---- aggregation: contiguous group per region, banks
                # visited consecutively so no accumulate lands after a
                # foreign start= armed its bank ----
                for ft in range(4):
                    for bt in range(2):
                        for idx, u in plan[(vn, bt)]:
                            nc.tensor.matmul(
                                agg_ps[:, ft, bt * 128:bt * 128 + 128],
                                msb[:, u, bt, ft * 128:ft * 128 + 128],
                                msk[:, idx, :],
                                start=(u == 0), stop=(u == vn - 1))

                # ---- stage next GRU rhs (fp8 SBUF) ----
                hT = new_hT(vn)
                nc.vector.tensor_copy(hT[:, 0, :, :], agg_ps[:, 0:2, :])
                nc.vector.tensor_copy(hT[:, 1, 0, :], agg_ps[:, 2, :])
                nc.vector.tensor_copy(hT[:117, 1, 1, :], agg_ps[:117, 3, :])
                if DBG and v == DBGV:
                    nc.sync.dma_start(out=d_dbg["dbg_ht1"], in_=hT[:])
                agg_prev = agg_ps

    nc.compile()
    return nc


def _quant8(x):
    import ml_dtypes
    return np.asarray(x, np.float32).astype(ml_dtypes.float8_e4m3)


def _prep_static(w_ih, w_hh, b_ih, b_hh, gate_w, gate_b, map_w,
                 fc1_w, fc1_b, fc2_w, fc2_b):
    import ml_dtypes
    f32 = np.float32
    bf16 = ml_dtypes.bfloat16

    # K-row layout of the GRU rhs h~: 0:501 h (SM-scaled), 501:509 x one-hot,
    # 509 const-1 (GRU bias), 510/511 zero. Weight column scaling: h-cols
    # x SWH (rhs carries SM) -> psum = SWH*SM*(W h); x/bias cols x SWH*SM.
    SXB = SWH * SM

    def gru_w(w_h, w_x, bias):
        W = np.zeros((512, 512), f32)
        W[0:501, 0:501] = w_h.T * SWH
        if w_x is not None:
            W[501:509, 0:501] = w_x.T * SXB
        W[509, 0:501] = bias * SXB
        return W

    bias_rz = (b_ih + b_hh).astype(f32)
    WRZ = np.stack([
        gru_w(w_hh[0:501], w_ih[0:501], bias_rz[0:501]),
        gru_w(w_hh[501:1002], w_ih[501:1002], bias_rz[501:1002])])
    WC = gru_w(w_hh[1002:1503], None, b_hh[1002:1503])
    WB0 = np.zeros((128, 512), f32)
    WB0[117:125, 0:501] = w_ih[1002:1503].T * SXB
    WB0[125, 0:501] = b_ih[1002:1503] * SXB
    WBq = np.asarray(_quant8(WB0), f32)
    WB = np.stack([WBq, WB0 - WBq])      # [hi/lo, 128, 512]

    WGM = np.zeros((2, 512, 512), f32)
    WGM[0, 0:501, 0:501] = gate_w[:, 0:501].T * SG
    WGM[1, 0:501, 0:501] = map_w[:, 0:501].T * SG

    # DoubleRow flats: [kpart, kp, plane, mt, ki, mpart]. For WC and WGM,
    # kp planes 2,3 hold the fp8 quantization residual (contracted against
    # the same rhs k-pairs), recovering near-bf16 weight precision.
    def with_residual(W):
        q = np.asarray(_quant8(W), f32)
        return np.concatenate([q, W - q], axis=-2)   # K axis doubled

    wrz = _quant8(WRZ.reshape(2, 2, 2, 128, 4, 128)
                  .transpose(3, 1, 0, 4, 2, 5).reshape(128, 4096))
    WGMr = with_residual(WGM)                        # [2gm, 1024K, 512]
    wgm = _quant8(WGMr.reshape(2, 4, 2, 128, 4, 128)
                  .transpose(3, 1, 0, 4, 2, 5).reshape(128, 8192))
    WCr = with_residual(WC)                          # [1024K, 512]
    wc = _quant8(WCr.reshape(4, 2, 128, 4, 128)
                 .transpose(2, 0, 3, 1, 4).reshape(128, 4096))
    wb = _quant8(WB.transpose(1, 0, 2).reshape(128, 1024))

    WF = np.zeros((512, 112), f32)
    WF[0:501, 0:56] = fc1_w.T
    WF[0:501, 56:112] = fc2_w.T
    wf = np.ascontiguousarray(
        WF.reshape(4, 128, 112).transpose(1, 0, 2).reshape(128, 448)
    ).astype(bf16)
    fcb = np.zeros((128, 1), f32)
    fcb[0:56, 0] = fc1_b
    fcb[56:112, 0] = fc2_b

    # per-step gate/mapper bias rows with fp8 residual:
    # [1, v, kp-sel(hi/lo), gm, mt, 128] (SG-scaled)
    gbr = np.zeros((MAX_N, 2, 2, 4, 128), f32)
    for v in range(MAX_N):
        gb_full = np.zeros(512, f32)
        mb_full = np.zeros(512, f32)
        gb_full[0:501] = (gate_b + gate_w[:, HS + v]) * SG
        mb_full[0:501] = map_w[:, HS + v] * SG
        for gm, row in ((0, gb_full), (1, mb_full)):
            hi = np.asarray(_quant8(row), f32)
            gbr[v, 0, gm] = hi.reshape(4, 128)
            gbr[v, 1, gm] = (row - hi).reshape(4, 128)
    gbr = _quant8(gbr.reshape(1, MAX_N * 2048))

    one = np.ones((1, 256), f32)
    id16 = np.eye(128, dtype=f32).astype(bf16)
    return dict(wrz=wrz, wc=wc, wb=wb, wgm=wgm, wf=wf, fcb=fcb,
                gbr=gbr, onerow=_quant8(one), id16=id16)


def _prep_core(node_types, adj, core):
    import ml_dtypes
    f32 = np.float32
    plan, NMSK = _mask_plan()
    off = core * BL
    nt = node_types[off:off + BL]          # [256, 16] int32
    ad = adj[off:off + BL].astype(f32)     # [256, 16, 16]

    xh = np.zeros((128, MAX_N, 256), f32)
    for bt in range(2):
        nb = nt[bt * 128:(bt + 1) * 128]   # [128, 16]
        for t in range(NVT):
            xh[117 + t, :, bt * 128:(bt + 1) * 128] = (nb.T == t)
    xh[125, :, :] = 1.0

    msk = np.zeros((128, NMSK, 128), f32)
    di = np.arange(128)
    for (vn, bt), ent in plan.items():
        ab = ad[bt * 128:(bt + 1) * 128]   # [128, 16, 16]
        for idx, u in ent:
            msk[di, idx, di] = ab[:, vn, u]
    return dict(xh=_quant8(xh.reshape(128, MAX_N * 256)),
                msk=msk.reshape(128, NMSK * 128).astype(ml_dtypes.bfloat16))


def kernel(node_types, adj, w_ih, w_hh, b_ih, b_hh, gate_w, gate_b, map_w,
           fc1_w, fc1_b, fc2_w, fc2_b):
    from concourse.bass_utils import run_bass_kernel_spmd

    if "nc" not in _CACHE:
        _CACHE["nc"] = _build_nc()
    nc = _CACHE["nc"]

    node_types = np.asarray(node_types)
    adj = np.asarray(adj, dtype=np.float32)
    static = _prep_static(
        np.asarray(w_ih, np.float32), np.asarray(w_hh, np.float32),
        np.asarray(b_ih, np.float32), np.asarray(b_hh, np.float32),
        np.asarray(gate_w, np.float32), np.asarray(gate_b, np.float32),
        np.asarray(map_w, np.float32),
        np.asarray(fc1_w, np.float32), np.asarray(fc1_b, np.float32),
        np.asarray(fc2_w, np.float32), np.asarray(fc2_b, np.float32))
    in_maps = []
    for c in range(NC_CORES):
        m = dict(static)
        m.update(_prep_core(node_types, adj, c))
        in_maps.append(m)

    res = run_bass_kernel_spmd(nc, in_maps, core_ids=list(range(NC_CORES)))
    ys = [res.results[c]["y"] for c in range(NC_CORES)]   # each [112, 256]
    out = np.concatenate(ys, axis=1).T                     # [2048, 112]
    return np.ascontiguousarray(out.astype(np.float32))
